# revision 2
# baseline (speedup 1.0000x reference)
"""Biquad lowpass filter (torchaudio-style) as a Trainium2 Bass kernel.

Math: the reference applies a 2nd-order IIR (biquad) per waveform:
    u[t] = b0 x[t] + b1 x[t-1] + b2 x[t-2]
    y[t] = u[t] - a1 y[t-1] - a2 y[t-2];  out = clip(y, -1, 1)

The poles have magnitude sqrt(a2) ~= 0.642, so the impulse response decays
below fp32 resolution within ~50 taps. Truncating at K=64 taps turns the
whole filter into a 64-tap FIR conv (tail error ~1e-12, far below the fp32
noise of the reference itself; validated vs a float64 scan).

Implementation per core (8 waveforms x 262144 samples):
  - View each waveform as 2048 blocks of 128 samples: X[k, j] = x[128 j + k].
  - y-block j = A^T X[:, j] + B^T X[:, j-1], where A/B are constant 128x128
    Toeplitz bands of the 64 taps -> two accumulating PE matmuls per block.
  - The moving operand is A/B (constant); the *stationary* operand is a
    128-column window of X, chosen with a column stride of 16 so the PSUM
    output lands directly in natural waveform layout (partition p = samples
    [2048 p, 2048 p + 2048)). Only the input needs a transpose (on PE);
    output DMA is fully contiguous.
"""

import numpy as np

N_CORES = 8
B_FULL = 64
T = 262144
WF_PER_CORE = B_FULL // N_CORES  # 8
NBLK = T // 128  # 2048 blocks of 128 samples per waveform
NGRP = NBLK // 128  # 16 block-groups per waveform
K_TAPS = 64

SAMPLE_RATE = 22050.0
CUTOFF = 0.4 * SAMPLE_RATE
Q = 0.707


def _impulse_response():
    """Truncated biquad impulse response, computed in float64."""
    w0 = 2.0 * np.pi * CUTOFF / SAMPLE_RATE
    alpha = np.sin(w0) / (2.0 * Q)
    cosw = np.cos(w0)
    a0 = 1.0 + alpha
    b0 = (1.0 - cosw) / 2.0 / a0
    b1 = (1.0 - cosw) / a0
    b2 = b0
    a1 = -2.0 * cosw / a0
    a2 = (1.0 - alpha) / a0
    g = np.zeros(K_TAPS)
    xi = np.zeros(K_TAPS)
    xi[0] = 1.0
    y1 = y2 = 0.0
    for t in range(K_TAPS):
        u = b0 * xi[t]
        if t >= 1:
            u += b1 * xi[t - 1]
        if t >= 2:
            u += b2 * xi[t - 2]
        y = u - a1 * y1 - a2 * y2
        g[t] = y
        y2, y1 = y1, y
    return g.astype(np.float32)


def _toeplitz_mats():
    """A[k, i] = g[i-k] (same-block taps), B[k, i] = g[128+i-k] (prev block)."""
    g = _impulse_response()
    A = np.zeros((128, 128), dtype=np.float32)
    B = np.zeros((128, 128), dtype=np.float32)
    for i in range(128):
        lo = max(0, i - K_TAPS + 1)
        A[lo : i + 1, i] = g[i - lo :: -1][: i - lo + 1]
        # B: k2 = 128 + i - k in (0, K_TAPS)
        for k in range(max(0, 128 + i - K_TAPS + 1), 128):
            k2 = 128 + i - k
            if 0 < k2 < K_TAPS:
                B[k, i] = g[k2]
    # rebuild A straightforwardly to avoid slicing mistakes
    A[:] = 0.0
    for i in range(128):
        for k in range(128):
            d = i - k
            if 0 <= d < K_TAPS:
                A[k, i] = g[d]
    return A, B


_BUILD_CACHE = {}


def _build_nc():
    if "nc" in _BUILD_CACHE:
        return _BUILD_CACHE["nc"]

    import concourse.bacc as bacc
    import concourse.mybir as mybir
    import concourse.tile as tile

    f32 = mybir.dt.float32
    nc = bacc.Bacc("TRN2", target_bir_lowering=False, debug=False)

    x_d = nc.dram_tensor("x", [WF_PER_CORE, T], f32, kind="ExternalInput")
    y_d = nc.dram_tensor("y", [WF_PER_CORE, T], f32, kind="ExternalOutput")
    a_d = nc.dram_tensor("Am", [128, 128], f32, kind="ExternalInput")
    b_d = nc.dram_tensor("Bm", [128, 128], f32, kind="ExternalInput")
    i_d = nc.dram_tensor("Im", [128, 128], f32, kind="ExternalInput")

    with tile.TileContext(nc) as tc:
        with (
            tc.tile_pool(name="const", bufs=1) as constp,
            tc.tile_pool(name="xnat", bufs=3) as xnatp,
            tc.tile_pool(name="xbuf", bufs=2) as xbufp,
            tc.tile_pool(name="sout", bufs=2) as soutp,
            tc.tile_pool(name="pt", bufs=2, space="PSUM") as ptp,
            tc.tile_pool(name="pc", bufs=2, space="PSUM") as pcp,
        ):
            a_sb = constp.tile([128, 128], f32, tag="a_sb")
            b_sb = constp.tile([128, 128], f32, tag="b_sb")
            i_sb = constp.tile([128, 128], f32, tag="i_sb")
            nc.sync.dma_start(out=a_sb[:, :], in_=a_d.ap())
            nc.sync.dma_start(out=b_sb[:, :], in_=b_d.ap())
            nc.sync.dma_start(out=i_sb[:, :], in_=i_d.ap())

            for w in range(WF_PER_CORE):
                # 1) load waveform in natural layout: partition p = x[2048p : 2048p+2048)
                xn = xnatp.tile([128, NBLK], f32, tag="xn")
                nc.sync.dma_start(
                    out=xn[:, :],
                    in_=x_d.ap()[w].rearrange("(p f) -> p f", p=128),
                )

                # 2) transpose into block layout with a +1 zero-pad column:
                #    xb[k, 1 + j] = X[k, j] = x[128 j + k]
                xb = xbufp.tile([128, NBLK + 16], f32, tag="xb")
                nc.gpsimd.memset(xb[:, 0:1], 0.0)
                for cb in range(4):
                    pt = ptp.tile([128, 512], f32, tag="pt")
                    for q in range(4):
                        c = 4 * cb + q
                        nc.tensor.transpose(
                            pt[:, 128 * q : 128 * (q + 1)],
                            xn[:, 128 * c : 128 * (c + 1)],
                            i_sb[:, :],
                        )
                    # pt[k, 128 q + p] = X[k, 16 p + (4 cb + q)] -> xb col 1 + 16p + 4cb + q
                    src = pt[:, :].rearrange("a (q i) -> a q i", q=4)
                    dst = xb[:, 1 + 4 * cb : 1 + 4 * cb + 2048].rearrange(
                        "a (p s) -> a s p", s=16
                    )[:, 0:4, :]
                    nc.scalar.copy(out=dst, in_=src)

                # 3) conv: group c covers blocks {16 m + c}; stationary = strided
                #    window of xb so PSUM partition m = waveform partition m.
                so = soutp.tile([128, NBLK], f32, tag="so")
                for kb in range(4):
                    pc = pcp.tile([128, 512], f32, tag="pc")
                    for q in range(4):
                        c = 4 * kb + q
                        a_lhsT = xb[:, 1 + c : 1 + c + 16 * 128 : 16]
                        b_lhsT = xb[:, c : c + 16 * 128 : 16]
                        nc.tensor.matmul(
                            pc[:, 128 * q : 128 * (q + 1)],
                            a_lhsT,
                            a_sb[:, :],
                            start=True,
                            stop=False,
                        )
                        nc.tensor.matmul(
                            pc[:, 128 * q : 128 * (q + 1)],
                            b_lhsT,
                            b_sb[:, :],
                            start=False,
                            stop=True,
                        )
                    # 4) clip to [-1, 1] while draining PSUM -> SBUF
                    nc.vector.tensor_scalar(
                        out=so[:, 512 * kb : 512 * (kb + 1)],
                        in0=pc[:, :],
                        scalar1=-1.0,
                        scalar2=1.0,
                        op0=mybir.AluOpType.max,
                        op1=mybir.AluOpType.min,
                    )

                # 5) store (fully contiguous per partition)
                nc.sync.dma_start(
                    out=y_d.ap()[w].rearrange("(p f) -> p f", p=128),
                    in_=so[:, :],
                )

    nc.finalize()
    _BUILD_CACHE["nc"] = nc
    return nc


def kernel(inputs: np.ndarray) -> np.ndarray:
    from concourse.bass_utils import run_bass_kernel_spmd

    x = np.ascontiguousarray(np.asarray(inputs, dtype=np.float32))
    assert x.shape == (B_FULL, T), x.shape

    A, B = _toeplitz_mats()
    ident = np.eye(128, dtype=np.float32)

    nc = _build_nc()
    in_maps = []
    for core in range(N_CORES):
        shard = np.ascontiguousarray(x[core * WF_PER_CORE : (core + 1) * WF_PER_CORE])
        in_maps.append({"x": shard, "Am": A, "Bm": B, "Im": ident})

    res = run_bass_kernel_spmd(nc, in_maps, core_ids=list(range(N_CORES)))
    out = np.concatenate([r["y"] for r in res.results], axis=0)
    return out.astype(np.float32)


if __name__ == "__main__":
    x = np.load("/tmp/x.npy")
    y = kernel(x)
    print("kernel output:", y.shape, y.dtype)
    np.save("/tmp/y_kernel.npy", y)
